# revision 7
# baseline (speedup 1.0000x reference)
"""Trainium2 Bass kernel for CTC beam-search decoding (nn_CTCPredictionsCpu).

Contract: kernel(data [128,64,32] f32, data_length [64] int32) -> preds [64,128] int32.

Strategy (pure data parallelism): 64 samples -> 8 cores x 8 samples.
Per core: 8 samples x 16 beams = 128 SBUF partitions, p = s*16 + i.

Device algorithm (per core):
  - log_softmax over classes, then "poison" frames t >= len(s):
    logp[blank]=0, others=NEG. Poisoned steps are provably identity updates,
    so no per-step length masking is needed.
  - CTC prefix beam search with per-beam state in partition p:
    lp_b, lp_nb, lp_tot, len, last, two rolling hashes (mod 8191) of the
    prefix, and the prefix chars themselves [128 cols].
  - prefix-merge detection via hash match on [i,j] beam pairs (j-state
    broadcast to all rows of a sample via a block-diagonal PE matmul).
  - top-16-of-528 selection: candidate values get their low 10 mantissa
    bits replaced by id = beam*33 + col, making all values distinct with
    deterministic arbitration; per-beam top-16 via max8/match_replace,
    then per-sample global top-16 the same way on a PE-broadcast 256-vec.
  - state update via a one-hot selection matrix SEL and PE gather matmuls.
"""
import os
import numpy as np

T, B, C = 128, 64, 32
BEAM = 16
BLANK = C - 1
NEG = np.float32(-1e30)
M = 8191.0
A1, A2 = 1031.0, 1537.0
NCORES = 8
SPC = B // NCORES          # samples per core = 8
P = 128                    # partitions
L = T                      # max prefix length

_cached = {}


def _build_nc(nsteps=T, debug=False):
    ABL = int(os.environ.get('ABL', '99'))
    import concourse.bass as bass
    import concourse.bacc as bacc
    import concourse.mybir as mybir
    from concourse import tile
    from concourse._compat import with_exitstack
    from contextlib import ExitStack

    f32 = mybir.dt.float32
    i32 = mybir.dt.int32
    Alu = mybir.AluOpType
    Act = mybir.ActivationFunctionType

    nc = bacc.Bacc("TRN2", target_bir_lowering=False, debug=False,
                   num_devices=NCORES)

    # ---- DRAM I/O ----
    d_data = nc.dram_tensor("data", [P, T, C], f32, kind="ExternalInput")
    d_lens = nc.dram_tensor("lens", [P, 1], f32, kind="ExternalInput")
    d_state0 = nc.dram_tensor("state0", [P, 16], f32, kind="ExternalInput")
    d_gv0 = nc.dram_tensor("gv0", [P, 16], f32, kind="ExternalInput")
    d_consts = nc.dram_tensor("consts", [P, 512], f32, kind="ExternalInput")
    d_out = nc.dram_tensor("preds", [SPC, T], i32, kind="ExternalOutput")
    d_dbg = {}
    if debug:
        for nm, sh in [("dstate", [P, 16]), ("dgv", [P, 16]),
                       ("dpref", [P, L]), ("dcg", [P, 35]),
                       ("dbcs", [P, 112]), ("dexps", [P, 4]),
                       ("dmt", [P, 16]), ("dv16", [P, 16]),
                       ("dpsg", [P, 169]), ("dlogp", [P, T * C]),
                       ("dlogpb", [P, T]), ("didxf", [P, 16]),
                       ("dval", [P, 1]), ("dcodef", [P, 1]),
                       ("dsnl", [P, 2])]:
            d_dbg[nm] = nc.dram_tensor(nm, sh, f32, kind="ExternalOutput")

    # consts layout (f32 cols):
    #  0:32   IOTA32 (0..31)
    #  32:65  IOTA33 (0..32)
    #  65:193 IOTA_L (0..127)
    #  193:321 BLKDIAG row (q-th row: 1.0 at cols of q's sample block)
    #  321:337 BEAMMASK (1 at col p%16)
    #  337:345 BLK8 (1 at col p//16)
    #  345:346 BEAM33 = 33*(p%16)
    #  346:347 ONESCOL = 1.0
    #  347:355 EVINIT row [0,0,0,0,0,0,NEG,0]
    #  355:356 NEGMS scratch? (unused)

    with tile.TileContext(nc) as tc, ExitStack() as ctx:
        pool = ctx.enter_context(tc.tile_pool(name="main", bufs=1))
        tpool = ctx.enter_context(tc.tile_pool(name="tmp", bufs=2))
        psum = ctx.enter_context(tc.tile_pool(name="ps", bufs=1, space="PSUM"))
        psum2 = ctx.enter_context(tc.tile_pool(name="ps2", bufs=1, space="PSUM"))

        # ---- persistent tiles ----
        LOGP = pool.tile([P, T, C], f32)     # becomes LOGP2 (col31=NEG) later
        LOGPB = pool.tile([P, T], f32)       # logp[...,31] per t
        CONS = pool.tile([P, 512], f32)
        STATE = pool.tile([P, 16], f32)
        # STATE cols: hA(0) hB(1) zeros(2) last(3) last2(4) lenm(5) lp_b(6)
        #             lp_nb(7) lp_tot(8) eg(9) pk3A(10) pk3B(11) pk3L(12)
        GV = pool.tile([P, 16], f32)         # coded global top16 (col0 ~ Ms)
        PREF = pool.tile([P, L], f32)
        EV = pool.tile([P, 8], f32)
        LENSM = pool.tile([P, 1], f32)

        IOTA32 = CONS[:, 0:32]
        IOTA33 = CONS[:, 32:65]
        IOTA_L = CONS[:, 65:193]
        BLKDIAG = CONS[:, 193:321]
        BEAMMASK = CONS[:, 321:337]
        BLK8 = CONS[:, 337:345]
        BEAM33 = CONS[:, 345:346]
        ONESCOL = CONS[:, 346:347]
        EVINIT = CONS[:, 347:355]
        BEAMIDX = CONS[:, 355:356]
        INM8INIT = CONS[:, 356:364]

        # ---- load inputs ----
        # state0/gv0 go through a DVE funnel copy so step-0 consumers
        # depend on one DVE sem instead of several DMA-queue sems
        # (hardware instructions have few sync-wait slots).
        STATE_raw = pool.tile([P, 16], f32)
        GV_raw = pool.tile([P, 16], f32)
        CONS_raw = pool.tile([P, 512], f32)
        LENS_raw = pool.tile([P, 1], f32)
        nc.sync.dma_start(LOGP[:], d_data[:])
        nc.sync.dma_start(CONS_raw[:], d_consts[:])
        nc.sync.dma_start(STATE_raw[:], d_state0[:])
        nc.sync.dma_start(GV_raw[:], d_gv0[:])
        nc.sync.dma_start(LENS_raw[:], d_lens[:])
        nc.vector.tensor_copy(STATE[:], STATE_raw[:])
        nc.vector.tensor_copy(GV[:], GV_raw[:])
        nc.vector.tensor_copy(CONS[:], CONS_raw[:])
        nc.vector.tensor_copy(LENSM[:], LENS_raw[:])
        nc.vector.tensor_copy(EV[:], EVINIT[:])
        INM8 = pool.tile([P, 8], f32)
        nc.vector.tensor_copy(INM8[:], INM8INIT[:])
        nc.vector.memset(PREF[:], 0.0)

        # ---- log_softmax over c for each (p, t); then poison ----
        MX = pool.tile([P, T], f32)
        SM = pool.tile([P, T], f32)
        lv = LOGP[:]
        nc.vector.tensor_reduce(MX[:], lv, axis=mybir.AxisListType.X,
                                op=Alu.max)
        mxb = MX[:].unsqueeze(2).broadcast_to([P, T, C])
        nc.vector.tensor_tensor(lv, lv, mxb, op=Alu.subtract)
        EXPV = pool.tile([P, T, C], f32)
        nc.scalar.activation(EXPV[:], lv, Act.Exp)
        nc.vector.tensor_reduce(SM[:], EXPV[:], axis=mybir.AxisListType.X,
                                op=Alu.add)
        nc.scalar.activation(SM[:], SM[:], Act.Ln)
        smb = SM[:].unsqueeze(2).broadcast_to([P, T, C])
        nc.vector.tensor_tensor(lv, lv, smb, op=Alu.subtract)

        # poison: for t >= len: all cols NEG, and LOGPB (blank col) = 0
        AM = pool.tile([P, T], f32)   # active mask: t < len
        nc.vector.tensor_scalar(AM[:], IOTA_L[:, 0:T], LENSM[:, 0:1], None,
                                op0=Alu.is_lt)
        l31 = LOGP[:][:, :, BLANK:BLANK+1]
        # LOGPB = active ? logp31 : 0   (extract BEFORE poisoning LOGP)
        nc.vector.tensor_tensor(LOGPB[:], l31.squeeze(2), AM[:], op=Alu.mult)
        # LOGP = logp*active + NEG*(1-active)
        IAM = pool.tile([P, T], f32)
        nc.vector.tensor_scalar(IAM[:], AM[:], -float(NEG), float(NEG),
                                op0=Alu.mult, op1=Alu.add)  # NEG*(1-active)
        amb = AM[:].unsqueeze(2).broadcast_to([P, T, C])
        nc.vector.tensor_tensor(lv, lv, amb, op=Alu.mult)
        iamb = IAM[:].unsqueeze(2).broadcast_to([P, T, C])
        nc.vector.tensor_tensor(lv, lv, iamb, op=Alu.add)
        # make LOGP2: col31 = NEG always (ext never uses blank)
        nc.vector.memset(l31.squeeze(2), float(NEG))

        # ---- main loop over T steps ----
        # temporaries
        tp = tpool

        def step(t_idx, dbg=False):
            LT = LOGP[:][:, t_idx, :]
            LBT = LOGPB[:, t_idx:t_idx+1]

            OH = tp.tile([P, C], f32, tag="OH")
            nc.vector.tensor_scalar(OH[:], IOTA32, STATE[:, 3:4], None,
                                    op0=Alu.is_equal)
            JK = tp.tile([P, C], f32, tag="JK")
            G = tp.tile([P, 1], f32, tag="G")
            nc.vector.tensor_tensor(JK[:], OH[:], LT, op=Alu.mult)
            nc.vector.tensor_reduce(G[:], JK[:], axis=mybir.AxisListType.X,
                                    op=Alu.add)
            # eg = exp(g) -> STATE col9
            nc.scalar.activation(STATE[:, 9:10], G[:], Act.Exp)

            if ABL <= 1:
                return
            # bc rhs build + matmul
            RHSA = tp.tile([P, 112], f32, tag="RHSA")
            s06 = STATE[:, 0:6].unsqueeze(2).broadcast_to([P, 6, 16])
            bm6 = BEAMMASK.unsqueeze(1).broadcast_to([P, 6, 16])
            nc.vector.tensor_tensor(
                RHSA[:, 0:96].rearrange("p (a b) -> p a b", a=6), s06, bm6,
                op=Alu.mult)
            nc.vector.tensor_scalar(RHSA[:, 96:112], BEAMMASK,
                                    STATE[:, 9:10], None, op0=Alu.mult)
            psBC = psum.tile([P, 112], f32, tag="psBC")
            nc.tensor.matmul(psBC[:], BLKDIAG, RHSA[:], start=True, stop=True)
            BCS = tp.tile([P, 112], f32, tag="BCS")
            nc.scalar.copy(BCS[:], psBC[:])
            # BCS: hA16(0:16) hB16(16:32) zeros16(32:48) last16(48:64)
            #      last2_16(64:80) lenm16(80:96) eg16(96:112)

            if ABL <= 2:
                return
            # candidates
            CG = tp.tile([P, 35], f32, tag="CG")
            # stay_b -> CG[:,0:1]
            nc.vector.tensor_scalar(CG[:, 0:1], LBT, STATE[:, 8:9], None,
                                    op0=Alu.add)
            ARG2 = tp.tile([P, 2], f32, tag="ARG2")
            nc.vector.tensor_tensor(ARG2[:, 0:1], STATE[:, 7:8], G[:],
                                    op=Alu.add)      # stay_nb_base
            nc.vector.tensor_copy(ARG2[:, 1:2], CG[:, 0:1])
            NEGMS = tp.tile([P, 1], f32, tag="NEGMS")
            nc.vector.tensor_scalar(NEGMS[:], GV[:, 0:1], -1.0, None,
                                    op0=Alu.mult)
            # NOTE: GV holds coded values; Ms off by <=1024 ulp - fine.
            EXPS = tp.tile([P, 4], f32, tag="EXPS")
            nc.scalar.activation(EXPS[:, 0:2], STATE[:, 6:9:2], Act.Exp,
                                 bias=NEGMS[:, 0:1])   # [elpb, elpt]
            nc.scalar.activation(EXPS[:, 2:4], ARG2[:], Act.Exp,
                                 bias=NEGMS[:, 0:1])   # [e_nb, e_b]
            DD = tp.tile([P, 1], f32, tag="DD")
            nc.vector.tensor_tensor(DD[:], STATE[:, 6:7], STATE[:, 8:9],
                                    op=Alu.subtract)
            EXTT = tp.tile([P, C], f32, tag="EXTT")
            nc.vector.tensor_scalar(EXTT[:], LT, STATE[:, 8:9], None,
                                    op0=Alu.add)
            EXTT2 = tp.tile([P, C], f32, tag="EXTT2")
            nc.vector.scalar_tensor_tensor(
                EXTT2[:], OH[:], DD[:, 0:1], EXTT[:],
                op0=Alu.mult, op1=Alu.add)
            DE = tp.tile([P, 1], f32, tag="DE")
            nc.vector.tensor_tensor(DE[:], EXPS[:, 0:1], EXPS[:, 1:2],
                                    op=Alu.subtract)

            if ABL <= 3:
                return
            # match & merge
            S1 = tp.tile([P, 48], f32, tag="S1")
            nc.vector.tensor_tensor(S1[:], BCS[:, 48:96], BCS[:, 0:48],
                                    op=Alu.subtract)
            V3 = tp.tile([P, 48], f32, tag="V3")
            pk3 = STATE[:, 10:13].unsqueeze(2).broadcast_to([P, 3, 16])
            nc.vector.tensor_tensor(
                V3[:].rearrange("p (a b) -> p a b", a=3),
                S1[:].rearrange("p (a b) -> p a b", a=3), pk3, op=Alu.add)
            V3I = tp.tile([P, 48], i32, tag="V3I")
            nc.vector.tensor_copy(V3I[:], V3[:])
            nc.vector.tensor_scalar(V3I[:], V3I[:], 8191, None,
                                    op0=Alu.bitwise_and)
            E3 = tp.tile([P, 48], f32, tag="E3")
            nc.vector.tensor_scalar(E3[:], V3I[:], 0, None, op0=Alu.is_equal)
            MT = tp.tile([P, 16], f32, tag="MT")
            nc.vector.tensor_tensor(MT[:], E3[:, 0:16], E3[:, 16:32],
                                    op=Alu.mult)
            nc.vector.tensor_tensor(MT[:], MT[:], E3[:, 32:48], op=Alu.mult)

            EQL = tp.tile([P, 16], f32, tag="EQL")
            nc.vector.tensor_scalar(EQL[:], BCS[:, 48:64], STATE[:, 3:4],
                                    None, op0=Alu.is_equal)
            Q16 = tp.tile([P, 16], f32, tag="Q16")
            nc.vector.scalar_tensor_tensor(
                Q16[:], EQL[:], DE[:, 0:1],
                EXPS[:, 1:2].broadcast_to([P, 16]),
                op0=Alu.mult, op1=Alu.add)
            CONTRIB = tp.tile([P, 16], f32, tag="CONTRIB")
            nc.vector.tensor_tensor(CONTRIB[:], MT[:], Q16[:], op=Alu.mult)
            nc.vector.tensor_tensor(CONTRIB[:], CONTRIB[:], BCS[:, 96:112],
                                    op=Alu.mult)

            if ABL <= 4:
                return
            # matchM -> transpose -> mrg matmul
            MM = tp.tile([P, 128], f32, tag="MM")
            mt8 = MT[:].unsqueeze(1).broadcast_to([P, 8, 16])
            blk8b = BLK8.unsqueeze(2).broadcast_to([P, 8, 16])
            nc.vector.tensor_tensor(
                MM[:].rearrange("p (a b) -> p a b", a=8), mt8, blk8b,
                op=Alu.mult)
            MMT = tp.tile([P, 128], f32, tag="MMT")
            nc.vector.transpose(MMT[:], MM[:])
            psMRG = psum.tile([P, C], f32, tag="psMRG")
            nc.tensor.matmul(psMRG[:], MMT[:], OH[:], start=True, stop=True)

            CM = tp.tile([P, 128], f32, tag="CM")
            cb8 = CONTRIB[:].unsqueeze(1).broadcast_to([P, 8, 16])
            nc.vector.tensor_tensor(
                CM[:].rearrange("p (a b) -> p a b", a=8), cb8, blk8b,
                op=Alu.mult)
            psS = psum.tile([P, 1], f32, tag="psS")
            nc.tensor.matmul(psS[:], CM[:], ONESCOL, start=True, stop=True)

            SNL = tp.tile([P, 2], f32, tag="SNL")
            nc.vector.tensor_tensor(SNL[:, 0:1], EXPS[:, 2:3], psS[:],
                                    op=Alu.add)
            nc.vector.tensor_tensor(SNL[:, 1:2], SNL[:, 0:1], EXPS[:, 3:4],
                                    op=Alu.add)
            LNS = tp.tile([P, 2], f32, tag="LNS")
            nc.scalar.activation(LNS[:], SNL[:], Act.Ln)
            nc.vector.tensor_scalar(CG[:, 1:3], LNS[:], GV[:, 0:1], -1e38,
                                    op0=Alu.add, op1=Alu.max)
            # ext2 = ext + mrg * -2e30 -> CG[:,3:35]
            nc.vector.scalar_tensor_tensor(
                CG[:, 3:35], psMRG[:], -2e30, EXTT2[:],
                op0=Alu.mult, op1=Alu.add)

            if ABL <= 5:
                return
            # ---- selection (exact values, positional arbitration) ----
            V16 = tp.tile([P, 16], f32, tag="V16")
            CX = tp.tile([P, 33], f32, tag="CX")
            nc.vector.max(V16[:, 0:8], CG[:, 2:35])
            nc.vector.match_replace(CX[:], V16[:, 0:8], CG[:, 2:35], -3e38)
            nc.vector.max(V16[:, 8:16], CX[:])

            RHSB = tp.tile([P, 256], f32, tag="RHSB")
            v16b = V16[:].unsqueeze(2).broadcast_to([P, 16, 16])
            bm16 = BEAMMASK.unsqueeze(1).broadcast_to([P, 16, 16])
            nc.vector.tensor_tensor(
                RHSB[:].rearrange("p (a b) -> p a b", a=16), v16b, bm16,
                op=Alu.mult)
            psBV = psum2.tile([P, 256], f32, tag="psBV")
            nc.tensor.matmul(psBV[:], BLKDIAG, RHSB[:], start=True, stop=True)
            BCV = tp.tile([P, 256], f32, tag="BCV")
            nc.scalar.copy(BCV[:], psBV[:])
            BCV2 = tp.tile([P, 256], f32, tag="BCV2")
            nc.vector.max(GV[:, 0:8], BCV[:])
            nc.vector.match_replace(BCV2[:], GV[:, 0:8], BCV[:], -3e38)
            nc.vector.max(GV[:, 8:16], BCV2[:])

            if ABL <= 6:
                return
            # positional source-id: claim positions r*16+j in bcV
            IDXG = tp.tile([P, 16], mybir.dt.uint32, tag="IDXG")
            nc.vector.max_index(IDXG[:, 0:8], GV[:, 0:8], BCV[:])
            nc.vector.max_index(IDXG[:, 8:16], GV[:, 8:16], BCV2[:])
            J16U = tp.tile([P, 16], mybir.dt.uint32, tag="J16U")
            nc.vector.tensor_scalar(J16U[:], IDXG[:], 15, None,
                                    op0=Alu.bitwise_and)
            J16 = tp.tile([P, 16], f32, tag="J16")
            nc.vector.tensor_copy(J16[:], J16U[:])
            FOUND = tp.tile([P, 16], f32, tag="FOUND")
            nc.vector.tensor_scalar(FOUND[:], J16[:], BEAMIDX[:, 0:1], None,
                                    op0=Alu.is_equal)
            SEL = tp.tile([P, 128], f32, tag="SEL")
            fnd8 = FOUND[:].unsqueeze(1).broadcast_to([P, 8, 16])
            nc.vector.tensor_tensor(
                SEL[:].rearrange("p (a b) -> p a b", a=8), fnd8, blk8b,
                op=Alu.mult)

            if ABL <= 7:
                return
            # gather matmuls: psG cols [0:6 state6 | 6:41 CG | 41:169 pref]
            psG = psum2.tile([P, 169], f32, tag="psG")
            nc.tensor.matmul(psG[:, 0:6], SEL[:], STATE[:, 0:6],
                             start=True, stop=True)
            nc.tensor.matmul(psG[:, 6:41], SEL[:], CG[:],
                             start=True, stop=True)
            nc.tensor.matmul(psG[:, 41:169], SEL[:], PREF[:],
                             start=True, stop=True)
            if ABL <= 8:
                return
            # ---- writeback ----
            # exact selected value: VAL[p] = GV[p, p%16]
            JK2 = tp.tile([P, 16], f32, tag="JK2")
            VAL = tp.tile([P, 1], f32, tag="VAL")
            nc.vector.tensor_tensor(JK2[:], GV[:], BEAMMASK, op=Alu.mult)
            nc.vector.tensor_reduce(VAL[:], JK2[:], axis=mybir.AxisListType.X,
                                    op=Alu.add)
            # c decode: find VAL in gathered ext row (first match), else stay
            EXT_SB = tp.tile([P, 32], f32, tag="EXT_SB")
            nc.scalar.copy(EXT_SB[:], psG[:, 9:41])
            nc.vector.tensor_copy(INM8[:, 0:1], VAL[:])
            IDX8 = tp.tile([P, 8], mybir.dt.uint32, tag="IDX8")
            nc.vector.max_index(IDX8[:], INM8[:], EXT_SB[:])
            IDXF1 = tp.tile([P, 1], f32, tag="IDXF1")
            nc.vector.tensor_copy(IDXF1[:], IDX8[:, 0:1])
            CODEF = tp.tile([P, 1], f32, tag="CODEF")
            nc.vector.tensor_scalar(CODEF[:], IDXF1[:], 1.0, None,
                                    op0=Alu.add)
            ISST = tp.tile([P, 1], i32, tag="ISST")
            nc.vector.tensor_scalar(ISST[:], IDXF1[:], 1e9, None,
                                    op0=Alu.is_gt)
            ISEX = tp.tile([P, 1], f32, tag="ISEX")
            nc.vector.tensor_scalar(ISEX[:], IDXF1[:], 1e9, None,
                                    op0=Alu.is_lt)
            # EV: [hA' hB' 0 c c len+1 NEG val]
            nc.vector.tensor_scalar(
                EV[:, 3:5], CODEF[:, 0:1].broadcast_to([P, 2]), -1.0, None,
                op0=Alu.add)
            nc.vector.tensor_scalar(EV[:, 5:6], psG[:, 5:6], 1.0, None,
                                    op0=Alu.add)
            TT0 = tp.tile([P, 2], f32, tag="TT0")
            nc.vector.scalar_tensor_tensor(
                TT0[:, 0:1], psG[:, 0:1], A1, CODEF[:], op0=Alu.mult,
                op1=Alu.add)
            nc.vector.scalar_tensor_tensor(
                TT0[:, 1:2], psG[:, 1:2], A2, CODEF[:], op0=Alu.mult,
                op1=Alu.add)
            TT0I = tp.tile([P, 2], i32, tag="TT0I")
            nc.vector.tensor_copy(TT0I[:], TT0[:])
            nc.vector.tensor_scalar(TT0I[:], TT0I[:], 8191, None,
                                    op0=Alu.bitwise_and)
            nc.vector.tensor_copy(EV[:, 0:2], TT0I[:])
            nc.vector.tensor_copy(EV[:, 7:8], VAL[:])
            # select into STATE[:,0:8]
            nc.vector.tensor_copy(STATE[:, 0:8], EV[:])
            nc.vector.copy_predicated(STATE[:, 0:8],
                                      ISST[:, 0:1].broadcast_to([P, 8]),
                                      psG[:, 0:8])
            nc.vector.tensor_copy(STATE[:, 8:9], VAL[:])
            nc.vector.tensor_scalar(STATE[:, 10:11], STATE[:, 0:1], A1, 1.0,
                                    op0=Alu.mult, op1=Alu.add)
            nc.vector.tensor_scalar(STATE[:, 11:12], STATE[:, 1:2], A2, 1.0,
                                    op0=Alu.mult, op1=Alu.add)
            nc.vector.tensor_scalar(STATE[:, 12:13], STATE[:, 5:6], -1.0,
                                    -1.0, op0=Alu.mult, op1=Alu.add)
            # prefix update
            MSK = tp.tile([P, L], i32, tag="MSK")
            nc.vector.scalar_tensor_tensor(
                MSK[:], IOTA_L, psG[:, 5:6],
                ISEX[:, 0:1].broadcast_to([P, L]),
                op0=Alu.is_equal, op1=Alu.mult)
            nc.scalar.copy(PREF[:], psG[:, 41:169])
            nc.vector.copy_predicated(PREF[:], MSK[:],
                                      STATE[:, 3:4].broadcast_to([P, L]))
            if dbg:
                nc.sync.dma_start(d_dbg["dcg"][:], CG[:])
                nc.sync.dma_start(d_dbg["dbcs"][:], BCS[:])
                nc.sync.dma_start(d_dbg["dexps"][:], EXPS[:])
                nc.sync.dma_start(d_dbg["dmt"][:], MT[:])
                nc.sync.dma_start(d_dbg["dv16"][:], V16[:])
                nc.sync.dma_start(d_dbg["dsnl"][:], SNL[:])
                DPS = tp.tile([P, 169], f32, tag="DPS")
                nc.vector.tensor_copy(DPS[:], psG[:])
                nc.sync.dma_start(d_dbg["dpsg"][:], DPS[:])
                nc.sync.dma_start(d_dbg["didxf"][:], J16[:])
                nc.sync.dma_start(d_dbg["dval"][:], VAL[:])
                nc.sync.dma_start(d_dbg["dcodef"][:], CODEF[:])

        for t_idx in range(nsteps):
            step(t_idx, dbg=(debug and t_idx == nsteps - 1))
        if debug:
            nc.sync.dma_start(d_dbg["dstate"][:], STATE[:])
            nc.sync.dma_start(d_dbg["dgv"][:], GV[:])
            nc.sync.dma_start(d_dbg["dpref"][:], PREF[:])
            nc.sync.dma_start(d_dbg["dlogp"][:], LOGP[:].rearrange(
                "p t c -> p (t c)"))
            nc.sync.dma_start(d_dbg["dlogpb"][:], LOGPB[:])

        # ---- output ----
        PM = pool.tile([P, L], f32)
        nc.vector.tensor_scalar(PM[:], IOTA_L, LENSM[:, 0:1], None,
                                op0=Alu.is_lt)
        nc.vector.tensor_tensor(PM[:], PM[:], PREF[:], op=Alu.mult)
        OUTI = pool.tile([P, L], i32)
        nc.vector.tensor_copy(OUTI[:], PM[:])
        # rows p = s*16 -> dram [8, 128]
        nc.sync.dma_start(
            d_out[:], OUTI[:].rearrange("(s b) l -> s b l", b=16)[:, 0:1, :])

    nc.compile()
    return nc


def _host_consts():
    iota32 = np.tile(np.arange(32, dtype=np.float32), (P, 1))
    iota33 = np.tile(np.arange(33, dtype=np.float32), (P, 1))
    iota_l = np.tile(np.arange(L, dtype=np.float32), (P, 1))
    blkdiag = np.zeros((P, 128), np.float32)
    for s in range(8):
        blkdiag[s*16:(s+1)*16, s*16:(s+1)*16] = 1.0
    beammask = np.zeros((P, 16), np.float32)
    beammask[np.arange(P), np.arange(P) % 16] = 1.0
    blk8 = np.zeros((P, 8), np.float32)
    blk8[np.arange(P), np.arange(P) // 16] = 1.0
    beam33 = (33.0 * (np.arange(P) % 16)).astype(np.float32)[:, None]
    ones = np.ones((P, 1), np.float32)
    evinit = np.zeros((P, 8), np.float32)
    evinit[:, 6] = NEG
    cons = np.zeros((P, 512), np.float32)
    cons[:, 0:32] = iota32
    cons[:, 32:65] = iota33
    cons[:, 65:193] = iota_l
    cons[:, 193:321] = blkdiag
    cons[:, 321:337] = beammask
    cons[:, 337:345] = blk8
    cons[:, 345:346] = beam33
    cons[:, 346:347] = ones
    cons[:, 347:355] = evinit
    cons[:, 355:356] = (np.arange(P) % 16).astype(np.float32)[:, None]
    cons[:, 356:364] = -3e38
    idc = ((np.arange(P) % 16)[:, None] * 33
           + np.arange(33)[None, :]).astype(np.int32)
    return cons, idc


def _host_state0():
    st = np.zeros((P, 16), np.float32)
    beam = np.arange(P) % 16
    st[:, 3] = -1.0   # last
    st[:, 4] = -1.0   # last2
    st[:, 5] = np.where(beam == 0, 0.0, -1000.0)  # len_m (invalid sentinel)
    st[:, 6] = np.where(beam == 0, 0.0, NEG)      # lp_b
    st[:, 7] = NEG                                # lp_nb
    st[:, 8] = np.where(beam == 0, 0.0, NEG)      # lp_tot
    st[:, 10] = 1.0   # pk3A = hA*A1+1
    st[:, 11] = 1.0   # pk3B
    st[:, 12] = -1.0  # pk3L = -len-1
    gv0 = np.full((P, 16), NEG, np.float32)
    gv0[:, 0] = 0.0   # Ms at step 0
    return st, gv0


def kernel(data, data_length):
    import sys
    if "/opt/trn_rl_repo" not in sys.path:
        sys.path.insert(0, "/opt/trn_rl_repo")
    from concourse.bass_utils import run_bass_kernel_spmd

    data = np.asarray(data, np.float32)
    lens = np.asarray(data_length)

    if "nc" not in _cached:
        _cached["nc"] = _build_nc()
        _cached["consts"] = _host_consts()
        _cached["state0"] = _host_state0()
    nc = _cached["nc"]
    cons, idc = _cached["consts"]
    st0, gv0 = _cached["state0"]

    in_maps = []
    for core in range(NCORES):
        s0 = core * SPC
        # [T, 8, C] -> beam-replicate -> [T, 128, C] -> [128, T, C]
        d = np.repeat(data[:, s0:s0+SPC, :], BEAM, axis=1)
        d = np.ascontiguousarray(d.transpose(1, 0, 2))
        lc = np.repeat(lens[s0:s0+SPC].astype(np.float32), BEAM)[:, None]
        in_maps.append({
            "data": d, "lens": np.ascontiguousarray(lc),
            "state0": st0, "gv0": gv0, "consts": cons,
        })

    _cached["last_in_maps"] = in_maps
    res = run_bass_kernel_spmd(nc, in_maps, list(range(NCORES)))
    out = np.zeros((B, T), np.int32)
    for core in range(NCORES):
        out[core*SPC:(core+1)*SPC] = res.results[core]["preds"]
    return out

